# revision 25
# baseline (speedup 1.0000x reference)
"""Causal self-attention (B=2, L=2048, C=2048, H=16) on 8 trn2 NeuronCores.

Sharding: tensor-parallel over heads - 2 heads per core. Each core computes
its heads' q/k/v from the full x, runs causal attention, and produces a
partial y @ w_proj[:, its-cols].T; the host sums the 8 partials (in f32).

Notes:
- The reference's RoPE rotates q and k by identical per-head (position-
  independent) angles; an orthogonal rotation applied to both sides leaves
  q.k unchanged, so RoPE is skipped entirely.
- ALL matmuls run in bf16 (PSUM accumulation stays f32). bf16 stationary
  tiles load in ~90ns vs ~204-384ns for fp32r, which removes the LDWEIGHTS
  bound on the projection matmuls and halves all DMA traffic.
- Inputs are pre-arranged on the host into the exact SBUF tile layout so
  every DMA moves contiguous multi-KB runs per partition (the naive [C, B*L]
  layout produced 1KB descriptors and descriptor-bound 60GB/s transfers).
- Softmax skips the max-subtraction (scores are ~N(0,1); exp is safe).
  Denominator: non-diagonal P tiles accumulate on the Vector engine (bf16
  2x adds) and hit the PE once per (head, q-group) via an all-ones
  [128,128] stationary matmul into a [128,512] PSUM tile; diagonal tiles
  accumulate directly on the PE. The reciprocal is full-width, so no
  gpsimd partition_broadcast is needed.
- Causal masking: scores matmuls only touch valid columns; the triangular
  128-col block of diagonal k-tiles is masked by one small DVE multiply.
- Phase-A PSUM evacuation runs on the Scalar engine (idle during phase A);
  proj evacuations run on Vector (Scalar is saturated by exp in phase B).
- Batch 0's tail projections are injected into batch 1's phase A as PE
  filler; output tiles are 2048 wide to cut DMA descriptor-gen 4x.
"""
import sys
sys.path.insert(0, '/opt/trn_rl_repo')
import contextlib
import ctypes
import os
import types

import numpy as np
import ml_dtypes

import concourse.bacc as bacc
import concourse.tile as tile
from concourse import mybir
from concourse.bass_utils import run_bass_kernel_spmd

F32 = mybir.dt.float32
BF16 = mybir.dt.bfloat16
AF = mybir.ActivationFunctionType

B, L, C, H, D = 2, 2048, 2048, 16, 128
NCORES = 8
HPC = H // NCORES            # heads per core
TC = 512                     # phase-A token chunk
NCH = L // TC                # chunks per batch
KT16 = C // 128              # contraction tiles over C
SCALE = 1.0 / float(np.sqrt(D))
NWARM = 24

LAST_RESULT = None           # BassKernelResults of the most recent run


def _install_ntff_shim():
    """Register the axon NTFF profile hook so BASS_TRACE=1 yields exec_time_ns."""
    if "antenv.axon_hooks" in sys.modules:
        return
    so_path = "/opt/axon/libaxon_pjrt.so"
    if not os.path.exists(so_path):
        return
    lib = ctypes.CDLL(so_path)
    if not hasattr(lib, "axon_start_nrt_profile"):
        return
    lib.axon_start_nrt_profile.argtypes = [ctypes.POINTER(ctypes.c_int64), ctypes.c_size_t]
    lib.axon_start_nrt_profile.restype = ctypes.c_int64
    lib.axon_stop_nrt_profile.argtypes = [ctypes.c_char_p]
    lib.axon_stop_nrt_profile.restype = ctypes.c_int64

    @contextlib.contextmanager
    def _hook(output_dir, device_ids):
        import jax
        jax.devices()
        if device_ids:
            ids = (ctypes.c_int64 * len(device_ids))(*device_ids)
            rc = lib.axon_start_nrt_profile(ids, len(device_ids))
        else:
            rc = lib.axon_start_nrt_profile(None, 0)
        if rc != 0:
            raise RuntimeError(f"axon_start_nrt_profile rc={rc}")
        try:
            yield
        finally:
            n = lib.axon_stop_nrt_profile(str(output_dir).encode())
            if n <= 0:
                print(f"ntff capture wrote {n} files to {output_dir}")

    mod = types.ModuleType("antenv.axon_hooks")
    mod.get_axon_ntff_profile_hook = lambda: _hook
    mod.set_axon_ntff_profile_hook = lambda h: None
    sys.modules["antenv.axon_hooks"] = mod


def _build():
    nc = bacc.Bacc()
    # All inputs pre-arranged: partition dim first, contiguous per partition.
    xH = nc.dram_tensor("xH", [128, B * NCH, KT16, TC], BF16, kind="ExternalInput")
    wqkH = nc.dram_tensor("wqkH", [128, 4, KT16, 128], BF16, kind="ExternalInput")
    wvH = nc.dram_tensor("wvH", [128, KT16, 2 * D], BF16, kind="ExternalInput")
    wpH = nc.dram_tensor("wpH", [128, 2, C], BF16, kind="ExternalInput")
    trid = nc.dram_tensor("trid", [128, 128], BF16, kind="ExternalInput")
    outd = nc.dram_tensor("out", [B * (L // 128), 128, C], BF16, kind="ExternalOutput")

    with tile.TileContext(nc) as tc:
        with tc.tile_pool(name="consts", bufs=1) as cp, \
             tc.tile_pool(name="big", bufs=1) as bp, \
             tc.tile_pool(name="xp", bufs=3) as xp, \
             tc.tile_pool(name="ptp", bufs=8) as ptp, \
             tc.tile_pool(name="smp", bufs=3) as smp, \
             tc.tile_pool(name="dnp", bufs=2) as dnp, \
             tc.tile_pool(name="osp", bufs=3) as osp, \
             tc.tile_pool(name="ps", bufs=2, space="PSUM") as ps, \
             tc.tile_pool(name="pyp", bufs=2, space="PSUM") as pyp, \
             tc.tile_pool(name="prp", bufs=2, space="PSUM") as prp, \
             tc.tile_pool(name="pop", bufs=2, space="PSUM") as pop:

            # PE warm-up on memset data: ramps the clock gate while the first
            # x chunk and weights stream in.
            warm = cp.tile([128, 256], BF16)
            nc.vector.memset(warm, 0.0)
            ones128 = cp.tile([128, 128], BF16)
            nc.vector.memset(ones128, 1.0)
            pw = ps.tile([128, 256], F32, tag="mm")
            for i in range(NWARM):
                nc.tensor.matmul(pw, warm[:, :128], warm,
                                 start=(i == 0), stop=(i == NWARM - 1))

            # Weights ride the sync hwdge queue: m=0 q/k, v, m=1 ahead of
            # chunk 0's x; the rest are emitted behind it (inside the loop).
            wqk = cp.tile([128, 4, KT16, 128], BF16)
            wv = cp.tile([128, KT16, 2 * D], BF16)
            wp = cp.tile([128, 2, C], BF16)
            tri = cp.tile([128, 128], BF16)
            xc0 = xp.tile([128, KT16, TC], BF16, tag="xc", name="xc0")
            nc.sync.dma_start(out=wqk[:, 0], in_=wqkH[:, 0])
            nc.sync.dma_start(out=xc0[:, 0:4], in_=xH[:, 0, 0:4])
            nc.sync.dma_start(out=xc0[:, 4:8], in_=xH[:, 0, 4:8])
            nc.sync.dma_start(out=xc0[:, 8:12], in_=xH[:, 0, 8:12])
            nc.sync.dma_start(out=xc0[:, 12:16], in_=xH[:, 0, 12:16])
            nc.sync.dma_start(out=wqk[:, 1], in_=wqkH[:, 1])
            nc.sync.dma_start(out=wqk[:, 2], in_=wqkH[:, 2])

            # Cross-batch carry of finished-but-unemitted proj pairs.
            pending_c = []
            otile = [None]
            evict_flip = [0]

            def proj_pair(bb, yTref, tt, nch, pool, alt=False):
                po = pool.tile([128, 512], F32, tag="po" if pool is pop else "mm")
                for hi in range(HPC):
                    nc.tensor.matmul(po, yTref[:, hi, tt * 128:(tt + 1) * 128],
                                     wp[:, hi, nch * 512:(nch + 1) * 512],
                                     start=(hi == 0), stop=(hi == HPC - 1))
                if nch == 0:
                    otile[0] = osp.tile([128, C], BF16, tag="ot", name="ot")
                oslice = otile[0][:, nch * 512:(nch + 1) * 512]
                if alt and evict_flip[0] % 2 == 0:
                    nc.scalar.copy(out=oslice, in_=po)
                else:
                    nc.vector.tensor_copy(oslice, po)
                evict_flip[0] += 1
                if nch == 3:
                    q = nc.gpsimd if (tt % 2 == 0) else nc.sync
                    q.dma_start(out=outd[bb * (L // 128) + tt], in_=otile[0])

            for b in range(B):
                # ---- Phase A: q/k/v projections for this batch ----
                QT = bp.tile([128, HPC, L], BF16, tag="QT")   # [d, hi, tok]
                KTt = bp.tile([128, HPC, L], BF16, tag="KT")
                V = bp.tile([128, L // 128, 2 * D], BF16, tag="V")  # [tok, tt, hi*D]
                for ch in range(NCH):
                    if b == 0 and ch == 0:
                        # chunk 0 was DMA'd before the loop; late weights
                        # ride the sync queue behind it
                        xc = xc0
                        nc.sync.dma_start(out=wqk[:, 3], in_=wqkH[:, 3])
                        nc.sync.dma_start(out=wv, in_=wvH[:, :, :])
                        nc.sync.dma_start(out=tri, in_=trid[:, :])
                        nc.sync.dma_start(out=wp, in_=wpH[:, :, :])
                    else:
                        xc = xp.tile([128, KT16, TC], BF16, tag="xc", name="xc")
                        nc.sync.dma_start(out=xc[:, :KT16 // 2],
                                          in_=xH[:, b * NCH + ch, :KT16 // 2])
                        nc.sync.dma_start(out=xc[:, KT16 // 2:],
                                          in_=xH[:, b * NCH + ch, KT16 // 2:])
                    for m in range(4):
                        pq = ps.tile([128, TC], F32, tag="mm")
                        for k in range(KT16):
                            nc.tensor.matmul(pq, wqk[:, m, k], xc[:, k],
                                             start=(k == 0), stop=(k == KT16 - 1))
                        dst = QT if m < 2 else KTt
                        dsl = dst[:, m % 2, ch * TC:(ch + 1) * TC]
                        if m % 2 == 0:
                            nc.scalar.copy(out=dsl, in_=pq)
                        else:
                            nc.vector.tensor_copy(dsl, pq)
                        if pending_c:
                            proj_pair(*pending_c.pop(0), pop, alt=True)
                    for tt in range(TC // 128):
                        # pyp's banks are idle during phase A; using them for
                        # pv gives phase A 4 rotating banks
                        pv = pyp.tile([128, 2 * D], F32, tag="acc", name="pv")
                        for k in range(KT16):
                            nc.tensor.matmul(pv, xc[:, k, tt * 128:(tt + 1) * 128],
                                             wv[:, k], start=(k == 0), stop=(k == KT16 - 1))
                        nc.vector.tensor_copy(V[:, ch * (TC // 128) + tt], pv)
                        if pending_c:
                            proj_pair(*pending_c.pop(0), pop, alt=True)

                # ---- Phase B: causal attention per head ----
                yT = bp.tile([128, HPC, L], BF16, tag="yT")   # [d, hi, tok]
                # Two heads interleaved so the PE always has an independent
                # chain while ACT computes exp for the other head. S^T matmuls
                # run 2 items ahead. Phase C (proj) pairs are injected as PE
                # filler one q-group after their yT inputs were normalized.
                items = [(hi, g, kt)
                         for g in range(4)
                         for kt in range(4 * (g + 1))
                         for hi in range(HPC)]

                def s_matmul(hi, g, kt):
                    off = max(0, 128 * (kt - 4 * g))
                    pss = ps.tile([128, 512], F32, tag="mm")
                    nc.tensor.matmul(pss[:, off:], KTt[:, hi, kt * 128:(kt + 1) * 128],
                                     QT[:, hi, g * 512 + off:(g + 1) * 512],
                                     start=True, stop=True)
                    return pss

                pss_q = [s_matmul(*items[0]), s_matmul(*items[1])]
                psy = {}
                psr = {}
                den = {}
                done_heads = {}
                for i, (hi, g, kt) in enumerate(items):
                    nkt = 4 * (g + 1)
                    off = max(0, 128 * (kt - 4 * g))
                    diag = kt >= 4 * g
                    if kt == 0:
                        psy[hi] = pyp.tile([128, 512], F32, tag="acc", name=f"psy{hi}")
                        psr[hi] = prp.tile([128, 512], F32, tag="rs", name=f"psr{hi}")
                    pss = pss_q.pop(0)
                    if kt == 0:
                        # exp writes straight into the den accumulator; psy
                        # reads the same buffer (saves a DVE copy)
                        den[hi] = dnp.tile([128, 512], BF16, tag=f"den{hi}",
                                           name=f"den{hi}")
                        ptile = den[hi]
                    else:
                        ptile = ptp.tile([128, 512], BF16)
                    nc.scalar.activation(ptile[:, off:], pss[:, off:], AF.Exp, scale=SCALE)
                    if i + 2 < len(items):
                        pss_q.append(s_matmul(*items[i + 2]))
                    if diag and g > 0 and kt == 4 * g:
                        # denominator of the whole non-diag prefix in one matmul
                        nc.tensor.matmul(psr[hi], ones128, den[hi],
                                         start=True, stop=False, skip_group_check=True)
                    if diag:
                        # triangular block of the diagonal k-tile only
                        nc.vector.tensor_mul(ptile[:, off:off + 128],
                                             ptile[:, off:off + 128], tri)
                    nc.tensor.matmul(psy[hi][:, off:], V[:, kt, hi * D:(hi + 1) * D],
                                     ptile[:, off:],
                                     start=(kt == 0), stop=(kt == nkt - 1),
                                     skip_group_check=True)
                    if diag:
                        nc.tensor.matmul(psr[hi][:, off:], ones128, ptile[:, off:],
                                         start=(g == 0 and kt == 0),
                                         stop=(kt == nkt - 1),
                                         skip_group_check=True)
                    elif kt > 0:
                        # running sum of non-diag P tiles off the PE (DVE
                        # bf16 SBUF adds run in 2x mode, ~330ns)
                        nc.vector.tensor_add(den[hi], den[hi], ptile)
                    if kt == nkt - 1:
                        rb = smp.tile([128, 512], F32, tag="rb")
                        nc.vector.reciprocal_approx_fast(out=rb, in_=psr[hi])
                        nc.vector.tensor_mul(yT[:, hi, g * 512:(g + 1) * 512],
                                             psy[hi], rb)
                        done_heads[g] = done_heads.get(g, 0) + 1
                        if done_heads[g] == HPC:
                            pending_c.extend((b, yT, tt, nch)
                                             for tt in range(4 * g, 4 * g + 4)
                                             for nch in range(4))
                    elif pending_c:
                        proj_pair(*pending_c.pop(0), pop)
                if b == B - 1:
                    # final drain: alternate the po ring with the (now idle)
                    # pss ring so evictions double-buffer deeper
                    for j, pc in enumerate(pending_c):
                        proj_pair(*pc, pop if j % 2 == 0 else ps, alt=True)
                    pending_c = []
    nc.compile()
    return nc


_cached_nc = None


def kernel(x, w_attn, w_proj):
    global _cached_nc, LAST_RESULT
    if os.environ.get("BASS_TRACE"):
        _install_ntff_shim()
    if _cached_nc is None:
        _cached_nc = _build()
    nc = _cached_nc

    x = np.asarray(x, dtype=np.float32)
    w_attn = np.asarray(w_attn, dtype=np.float32)
    w_proj = np.asarray(w_proj, dtype=np.float32)

    bf16 = ml_dtypes.bfloat16
    # [p, b, ch, k, tc] layout: contiguous 16KB per partition per chunk
    xHa = np.ascontiguousarray(
        x.reshape(B, NCH, TC, KT16, 128).transpose(4, 0, 1, 3, 2)
    ).astype(bf16).reshape(128, B * NCH, KT16, TC)
    kk = np.arange(128)[:, None]
    qq = np.arange(128)[None, :]
    tri = (kk <= qq).astype(bf16)

    in_maps = []
    for c in range(NCORES):
        h0 = HPC * c
        wq = w_attn[h0 * D:(h0 + HPC) * D]
        wk = w_attn[C + h0 * D: C + (h0 + HPC) * D]
        wvv = w_attn[2 * C + h0 * D: 2 * C + (h0 + HPC) * D]
        wqkT = np.concatenate([wq, wk], axis=0).T          # [C, 4D]
        wqkHa = np.ascontiguousarray(
            wqkT.reshape(KT16, 128, 4, 128).transpose(1, 2, 0, 3)
        ).astype(bf16)                                     # [p, m, k, j]
        wvHa = np.ascontiguousarray(
            wvv.T.reshape(KT16, 128, 2 * D).transpose(1, 0, 2)
        ).astype(bf16)                                     # [p, k, vcol]
        wpT = w_proj[:, h0 * D:(h0 + HPC) * D].T           # [2D, C]
        wpHa = np.ascontiguousarray(
            wpT.reshape(2, 128, C).transpose(1, 0, 2)
        ).astype(bf16)                                     # [p, hi, col]
        in_maps.append({
            "xH": xHa,
            "wqkH": wqkHa,
            "wvH": wvHa,
            "wpH": wpHa,
            "trid": tri,
        })

    res = run_bass_kernel_spmd(nc, in_maps, core_ids=list(range(NCORES)))
    LAST_RESULT = res
    acc = res.results[0]["out"].astype(np.float32)
    for i in range(1, NCORES):
        acc += res.results[i]["out"].astype(np.float32)
    return acc.reshape(B, L, C)


# revision 26
# speedup vs baseline: 1.0014x; 1.0014x over previous
"""Causal self-attention (B=2, L=2048, C=2048, H=16) on 8 trn2 NeuronCores.

Sharding: tensor-parallel over heads - 2 heads per core. Each core computes
its heads' q/k/v from the full x, runs causal attention, and produces a
partial y @ w_proj[:, its-cols].T; the host sums the 8 partials (in f32).

Notes:
- The reference's RoPE rotates q and k by identical per-head (position-
  independent) angles; an orthogonal rotation applied to both sides leaves
  q.k unchanged, so RoPE is skipped entirely.
- ALL matmuls run in bf16 (PSUM accumulation stays f32). bf16 stationary
  tiles load in ~90ns vs ~204-384ns for fp32r, which removes the LDWEIGHTS
  bound on the projection matmuls and halves all DMA traffic.
- Inputs are pre-arranged on the host into the exact SBUF tile layout so
  every DMA moves contiguous multi-KB runs per partition (the naive [C, B*L]
  layout produced 1KB descriptors and descriptor-bound 60GB/s transfers).
- Softmax skips the max-subtraction (scores are ~N(0,1); exp is safe).
  Denominator: non-diagonal P tiles accumulate on the Vector engine (bf16
  2x adds) and hit the PE once per (head, q-group) via an all-ones
  [128,128] stationary matmul into a [128,512] PSUM tile; diagonal tiles
  accumulate directly on the PE. The reciprocal is full-width, so no
  gpsimd partition_broadcast is needed.
- Causal masking: scores matmuls only touch valid columns; the triangular
  128-col block of diagonal k-tiles is masked by one small DVE multiply.
- Phase-A PSUM evacuation runs on the Scalar engine (idle during phase A);
  proj evacuations run on Vector (Scalar is saturated by exp in phase B).
- Batch 0's tail projections are injected into batch 1's phase A as PE
  filler; output tiles are 2048 wide to cut DMA descriptor-gen 4x.
"""
import sys
sys.path.insert(0, '/opt/trn_rl_repo')
import contextlib
import ctypes
import os
import types

import numpy as np
import ml_dtypes

import concourse.bacc as bacc
import concourse.tile as tile
from concourse import mybir
from concourse.bass_utils import run_bass_kernel_spmd

F32 = mybir.dt.float32
BF16 = mybir.dt.bfloat16
AF = mybir.ActivationFunctionType

B, L, C, H, D = 2, 2048, 2048, 16, 128
NCORES = 8
HPC = H // NCORES            # heads per core
TC = 512                     # phase-A token chunk
NCH = L // TC                # chunks per batch
KT16 = C // 128              # contraction tiles over C
SCALE = 1.0 / float(np.sqrt(D))
NWARM = 24

LAST_RESULT = None           # BassKernelResults of the most recent run


def _install_ntff_shim():
    """Register the axon NTFF profile hook so BASS_TRACE=1 yields exec_time_ns."""
    if "antenv.axon_hooks" in sys.modules:
        return
    so_path = "/opt/axon/libaxon_pjrt.so"
    if not os.path.exists(so_path):
        return
    lib = ctypes.CDLL(so_path)
    if not hasattr(lib, "axon_start_nrt_profile"):
        return
    lib.axon_start_nrt_profile.argtypes = [ctypes.POINTER(ctypes.c_int64), ctypes.c_size_t]
    lib.axon_start_nrt_profile.restype = ctypes.c_int64
    lib.axon_stop_nrt_profile.argtypes = [ctypes.c_char_p]
    lib.axon_stop_nrt_profile.restype = ctypes.c_int64

    @contextlib.contextmanager
    def _hook(output_dir, device_ids):
        import jax
        jax.devices()
        if device_ids:
            ids = (ctypes.c_int64 * len(device_ids))(*device_ids)
            rc = lib.axon_start_nrt_profile(ids, len(device_ids))
        else:
            rc = lib.axon_start_nrt_profile(None, 0)
        if rc != 0:
            raise RuntimeError(f"axon_start_nrt_profile rc={rc}")
        try:
            yield
        finally:
            n = lib.axon_stop_nrt_profile(str(output_dir).encode())
            if n <= 0:
                print(f"ntff capture wrote {n} files to {output_dir}")

    mod = types.ModuleType("antenv.axon_hooks")
    mod.get_axon_ntff_profile_hook = lambda: _hook
    mod.set_axon_ntff_profile_hook = lambda h: None
    sys.modules["antenv.axon_hooks"] = mod


def _build():
    nc = bacc.Bacc()
    # All inputs pre-arranged: partition dim first, contiguous per partition.
    xH = nc.dram_tensor("xH", [128, B * NCH, KT16, TC], BF16, kind="ExternalInput")
    wqkH = nc.dram_tensor("wqkH", [128, 4, KT16, 128], BF16, kind="ExternalInput")
    wvH = nc.dram_tensor("wvH", [128, KT16, 2 * D], BF16, kind="ExternalInput")
    wpH = nc.dram_tensor("wpH", [128, 2, C], BF16, kind="ExternalInput")
    trid = nc.dram_tensor("trid", [128, 128], BF16, kind="ExternalInput")
    outd = nc.dram_tensor("out", [B * (L // 128), 128, C], BF16, kind="ExternalOutput")

    with tile.TileContext(nc) as tc:
        with tc.tile_pool(name="consts", bufs=1) as cp, \
             tc.tile_pool(name="big", bufs=1) as bp, \
             tc.tile_pool(name="xp", bufs=3) as xp, \
             tc.tile_pool(name="ptp", bufs=8) as ptp, \
             tc.tile_pool(name="smp", bufs=3) as smp, \
             tc.tile_pool(name="dnp", bufs=2) as dnp, \
             tc.tile_pool(name="osp", bufs=3) as osp, \
             tc.tile_pool(name="ps", bufs=3, space="PSUM") as ps, \
             tc.tile_pool(name="pyp", bufs=2, space="PSUM") as pyp, \
             tc.tile_pool(name="prp", bufs=1, space="PSUM") as prp, \
             tc.tile_pool(name="pop", bufs=2, space="PSUM") as pop:

            # PE warm-up on memset data: ramps the clock gate while the first
            # x chunk and weights stream in.
            warm = cp.tile([128, 256], BF16)
            nc.vector.memset(warm, 0.0)
            ones128 = cp.tile([128, 128], BF16)
            nc.vector.memset(ones128, 1.0)
            pw = ps.tile([128, 256], F32, tag="mm")
            for i in range(NWARM):
                nc.tensor.matmul(pw, warm[:, :128], warm,
                                 start=(i == 0), stop=(i == NWARM - 1))

            # Weights ride the sync hwdge queue: m=0 q/k, v, m=1 ahead of
            # chunk 0's x; the rest are emitted behind it (inside the loop).
            wqk = cp.tile([128, 4, KT16, 128], BF16)
            wv = cp.tile([128, KT16, 2 * D], BF16)
            wp = cp.tile([128, 2, C], BF16)
            tri = cp.tile([128, 128], BF16)
            xc0 = xp.tile([128, KT16, TC], BF16, tag="xc", name="xc0")
            nc.sync.dma_start(out=wqk[:, 0], in_=wqkH[:, 0])
            nc.sync.dma_start(out=xc0[:, 0:4], in_=xH[:, 0, 0:4])
            nc.sync.dma_start(out=xc0[:, 4:8], in_=xH[:, 0, 4:8])
            nc.sync.dma_start(out=xc0[:, 8:12], in_=xH[:, 0, 8:12])
            nc.sync.dma_start(out=xc0[:, 12:16], in_=xH[:, 0, 12:16])
            nc.sync.dma_start(out=wqk[:, 1], in_=wqkH[:, 1])
            nc.sync.dma_start(out=wqk[:, 2], in_=wqkH[:, 2])

            # Cross-batch carry of finished-but-unemitted proj pairs.
            pending_c = []
            otile = [None]
            evict_flip = [0]

            def proj_pair(bb, yTref, tt, nch, pool, alt=False):
                po = pool.tile([128, 512], F32, tag="po" if pool is pop else "mm")
                for hi in range(HPC):
                    nc.tensor.matmul(po, yTref[:, hi, tt * 128:(tt + 1) * 128],
                                     wp[:, hi, nch * 512:(nch + 1) * 512],
                                     start=(hi == 0), stop=(hi == HPC - 1))
                if nch == 0:
                    otile[0] = osp.tile([128, C], BF16, tag="ot", name="ot")
                oslice = otile[0][:, nch * 512:(nch + 1) * 512]
                if alt and evict_flip[0] % 2 == 0:
                    nc.scalar.copy(out=oslice, in_=po)
                else:
                    nc.vector.tensor_copy(oslice, po)
                evict_flip[0] += 1
                if nch == 3:
                    q = nc.gpsimd if (tt % 2 == 0) else nc.sync
                    q.dma_start(out=outd[bb * (L // 128) + tt], in_=otile[0])

            for b in range(B):
                # ---- Phase A: q/k/v projections for this batch ----
                QT = bp.tile([128, HPC, L], BF16, tag="QT")   # [d, hi, tok]
                KTt = bp.tile([128, HPC, L], BF16, tag="KT")
                V = bp.tile([128, L // 128, 2 * D], BF16, tag="V")  # [tok, tt, hi*D]
                for ch in range(NCH):
                    if b == 0 and ch == 0:
                        # chunk 0 was DMA'd before the loop; late weights
                        # ride the sync queue behind it
                        xc = xc0
                        nc.sync.dma_start(out=wqk[:, 3], in_=wqkH[:, 3])
                        nc.sync.dma_start(out=wv, in_=wvH[:, :, :])
                        nc.sync.dma_start(out=tri, in_=trid[:, :])
                        nc.sync.dma_start(out=wp, in_=wpH[:, :, :])
                    else:
                        xc = xp.tile([128, KT16, TC], BF16, tag="xc", name="xc")
                        nc.sync.dma_start(out=xc[:, :KT16 // 2],
                                          in_=xH[:, b * NCH + ch, :KT16 // 2])
                        nc.sync.dma_start(out=xc[:, KT16 // 2:],
                                          in_=xH[:, b * NCH + ch, KT16 // 2:])
                    for m in range(4):
                        pq = ps.tile([128, TC], F32, tag="mm")
                        for k in range(KT16):
                            nc.tensor.matmul(pq, wqk[:, m, k], xc[:, k],
                                             start=(k == 0), stop=(k == KT16 - 1))
                        dst = QT if m < 2 else KTt
                        dsl = dst[:, m % 2, ch * TC:(ch + 1) * TC]
                        if m % 2 == 0:
                            nc.scalar.copy(out=dsl, in_=pq)
                        else:
                            nc.vector.tensor_copy(dsl, pq)
                        if pending_c:
                            proj_pair(*pending_c.pop(0), pop, alt=True)
                    for tt in range(TC // 128):
                        # pyp's banks are idle during phase A; using them for
                        # pv gives phase A 4 rotating banks
                        pv = pyp.tile([128, 2 * D], F32, tag="acc", name="pv")
                        for k in range(KT16):
                            nc.tensor.matmul(pv, xc[:, k, tt * 128:(tt + 1) * 128],
                                             wv[:, k], start=(k == 0), stop=(k == KT16 - 1))
                        nc.vector.tensor_copy(V[:, ch * (TC // 128) + tt], pv)
                        if pending_c:
                            proj_pair(*pending_c.pop(0), pop, alt=True)

                # ---- Phase B: causal attention per head ----
                yT = bp.tile([128, HPC, L], BF16, tag="yT")   # [d, hi, tok]
                # Two heads interleaved so the PE always has an independent
                # chain while ACT computes exp for the other head. S^T matmuls
                # run 2 items ahead. Phase C (proj) pairs are injected as PE
                # filler one q-group after their yT inputs were normalized.
                items = [(hi, g, kt)
                         for g in range(4)
                         for hi in range(HPC)
                         for kt in range(4 * (g + 1))]

                def s_matmul(hi, g, kt):
                    off = max(0, 128 * (kt - 4 * g))
                    pss = ps.tile([128, 512], F32, tag="mm")
                    nc.tensor.matmul(pss[:, off:], KTt[:, hi, kt * 128:(kt + 1) * 128],
                                     QT[:, hi, g * 512 + off:(g + 1) * 512],
                                     start=True, stop=True)
                    return pss

                pss_q = [s_matmul(*items[0]), s_matmul(*items[1]),
                         s_matmul(*items[2])]
                psy = {}
                psr = {}
                den = {}
                done_heads = {}
                for i, (hi, g, kt) in enumerate(items):
                    nkt = 4 * (g + 1)
                    off = max(0, 128 * (kt - 4 * g))
                    diag = kt >= 4 * g
                    if kt == 0:
                        psy[hi] = pyp.tile([128, 512], F32, tag="acc", name=f"psy{hi}")
                        psr[hi] = prp.tile([128, 512], F32, tag="rs", name=f"psr{hi}")
                    pss = pss_q.pop(0)
                    if kt == 0:
                        # exp writes straight into the den accumulator; psy
                        # reads the same buffer (saves a DVE copy)
                        den[hi] = dnp.tile([128, 512], BF16, tag=f"den{hi}",
                                           name=f"den{hi}")
                        ptile = den[hi]
                    else:
                        ptile = ptp.tile([128, 512], BF16)
                    nc.scalar.activation(ptile[:, off:], pss[:, off:], AF.Exp, scale=SCALE)
                    if i + 3 < len(items):
                        pss_q.append(s_matmul(*items[i + 3]))
                    if diag and g > 0 and kt == 4 * g:
                        # denominator of the whole non-diag prefix in one matmul
                        nc.tensor.matmul(psr[hi], ones128, den[hi],
                                         start=True, stop=False, skip_group_check=True)
                    if diag:
                        # triangular block of the diagonal k-tile only
                        nc.vector.tensor_mul(ptile[:, off:off + 128],
                                             ptile[:, off:off + 128], tri)
                    nc.tensor.matmul(psy[hi][:, off:], V[:, kt, hi * D:(hi + 1) * D],
                                     ptile[:, off:],
                                     start=(kt == 0), stop=(kt == nkt - 1),
                                     skip_group_check=True)
                    if diag:
                        nc.tensor.matmul(psr[hi][:, off:], ones128, ptile[:, off:],
                                         start=(g == 0 and kt == 0),
                                         stop=(kt == nkt - 1),
                                         skip_group_check=True)
                    elif kt > 0:
                        # running sum of non-diag P tiles off the PE (DVE
                        # bf16 SBUF adds run in 2x mode, ~330ns)
                        nc.vector.tensor_add(den[hi], den[hi], ptile)
                    if kt == nkt - 1:
                        rb = smp.tile([128, 512], F32, tag="rb")
                        nc.vector.reciprocal_approx_fast(out=rb, in_=psr[hi])
                        nc.vector.tensor_mul(yT[:, hi, g * 512:(g + 1) * 512],
                                             psy[hi], rb)
                        done_heads[g] = done_heads.get(g, 0) + 1
                        if done_heads[g] == HPC:
                            pending_c.extend((b, yT, tt, nch)
                                             for tt in range(4 * g, 4 * g + 4)
                                             for nch in range(4))
                    elif pending_c:
                        proj_pair(*pending_c.pop(0), pop)
                if b == B - 1:
                    # final drain: alternate the po ring with the (now idle)
                    # pss ring so evictions double-buffer deeper
                    for j, pc in enumerate(pending_c):
                        proj_pair(*pc, pop if j % 2 == 0 else ps, alt=True)
                    pending_c = []
    nc.compile()
    return nc


_cached_nc = None


def kernel(x, w_attn, w_proj):
    global _cached_nc, LAST_RESULT
    if os.environ.get("BASS_TRACE"):
        _install_ntff_shim()
    if _cached_nc is None:
        _cached_nc = _build()
    nc = _cached_nc

    x = np.asarray(x, dtype=np.float32)
    w_attn = np.asarray(w_attn, dtype=np.float32)
    w_proj = np.asarray(w_proj, dtype=np.float32)

    bf16 = ml_dtypes.bfloat16
    # [p, b, ch, k, tc] layout: contiguous 16KB per partition per chunk
    xHa = np.ascontiguousarray(
        x.reshape(B, NCH, TC, KT16, 128).transpose(4, 0, 1, 3, 2)
    ).astype(bf16).reshape(128, B * NCH, KT16, TC)
    kk = np.arange(128)[:, None]
    qq = np.arange(128)[None, :]
    tri = (kk <= qq).astype(bf16)

    in_maps = []
    for c in range(NCORES):
        h0 = HPC * c
        wq = w_attn[h0 * D:(h0 + HPC) * D]
        wk = w_attn[C + h0 * D: C + (h0 + HPC) * D]
        wvv = w_attn[2 * C + h0 * D: 2 * C + (h0 + HPC) * D]
        wqkT = np.concatenate([wq, wk], axis=0).T          # [C, 4D]
        wqkHa = np.ascontiguousarray(
            wqkT.reshape(KT16, 128, 4, 128).transpose(1, 2, 0, 3)
        ).astype(bf16)                                     # [p, m, k, j]
        wvHa = np.ascontiguousarray(
            wvv.T.reshape(KT16, 128, 2 * D).transpose(1, 0, 2)
        ).astype(bf16)                                     # [p, k, vcol]
        wpT = w_proj[:, h0 * D:(h0 + HPC) * D].T           # [2D, C]
        wpHa = np.ascontiguousarray(
            wpT.reshape(2, 128, C).transpose(1, 0, 2)
        ).astype(bf16)                                     # [p, hi, col]
        in_maps.append({
            "xH": xHa,
            "wqkH": wqkHa,
            "wvH": wvHa,
            "wpH": wpHa,
            "trid": tri,
        })

    res = run_bass_kernel_spmd(nc, in_maps, core_ids=list(range(NCORES)))
    LAST_RESULT = res
    acc = res.results[0]["out"].astype(np.float32)
    for i in range(1, NCORES):
        acc += res.results[i]["out"].astype(np.float32)
    return acc.reshape(B, L, C)


# revision 27
# speedup vs baseline: 1.0334x; 1.0320x over previous
"""Causal self-attention (B=2, L=2048, C=2048, H=16) on 8 trn2 NeuronCores.

Sharding: tensor-parallel over heads - 2 heads per core. Each core computes
its heads' q/k/v from the full x, runs causal attention, and produces a
partial y @ w_proj[:, its-cols].T; the host sums the 8 partials (in f32).

Notes:
- The reference's RoPE rotates q and k by identical per-head (position-
  independent) angles; an orthogonal rotation applied to both sides leaves
  q.k unchanged, so RoPE is skipped entirely.
- ALL matmuls run in bf16 (PSUM accumulation stays f32). bf16 stationary
  tiles load in ~90ns vs ~204-384ns for fp32r, which removes the LDWEIGHTS
  bound on the projection matmuls and halves all DMA traffic.
- Inputs are pre-arranged on the host into the exact SBUF tile layout so
  every DMA moves contiguous multi-KB runs per partition (the naive [C, B*L]
  layout produced 1KB descriptors and descriptor-bound 60GB/s transfers).
- Softmax skips the max-subtraction (scores are ~N(0,1); exp is safe).
  Denominator: non-diagonal P tiles accumulate on the Vector engine (bf16
  2x adds) and hit the PE once per (head, q-group) via an all-ones
  [128,128] stationary matmul into a [128,512] PSUM tile; diagonal tiles
  accumulate directly on the PE. The reciprocal is full-width, so no
  gpsimd partition_broadcast is needed.
- Causal masking: scores matmuls only touch valid columns; the triangular
  128-col block of diagonal k-tiles is masked by one small DVE multiply.
- Phase-A PSUM evacuation runs on the Scalar engine (idle during phase A);
  proj evacuations run on Vector (Scalar is saturated by exp in phase B).
- Batch 0's tail projections are injected into batch 1's phase A as PE
  filler; output tiles are 2048 wide to cut DMA descriptor-gen 4x.
"""
import sys
sys.path.insert(0, '/opt/trn_rl_repo')
import contextlib
import ctypes
import os
import types

import numpy as np
import ml_dtypes

import concourse.bacc as bacc
import concourse.tile as tile
from concourse import mybir
from concourse.bass_utils import run_bass_kernel_spmd

F32 = mybir.dt.float32
BF16 = mybir.dt.bfloat16
AF = mybir.ActivationFunctionType

B, L, C, H, D = 2, 2048, 2048, 16, 128
NCORES = 8
HPC = H // NCORES            # heads per core
TC = 512                     # phase-A token chunk
NCH = L // TC                # chunks per batch
KT16 = C // 128              # contraction tiles over C
SCALE = 1.0 / float(np.sqrt(D))
NWARM = 24

LAST_RESULT = None           # BassKernelResults of the most recent run


def _install_ntff_shim():
    """Register the axon NTFF profile hook so BASS_TRACE=1 yields exec_time_ns."""
    if "antenv.axon_hooks" in sys.modules:
        return
    so_path = "/opt/axon/libaxon_pjrt.so"
    if not os.path.exists(so_path):
        return
    lib = ctypes.CDLL(so_path)
    if not hasattr(lib, "axon_start_nrt_profile"):
        return
    lib.axon_start_nrt_profile.argtypes = [ctypes.POINTER(ctypes.c_int64), ctypes.c_size_t]
    lib.axon_start_nrt_profile.restype = ctypes.c_int64
    lib.axon_stop_nrt_profile.argtypes = [ctypes.c_char_p]
    lib.axon_stop_nrt_profile.restype = ctypes.c_int64

    @contextlib.contextmanager
    def _hook(output_dir, device_ids):
        import jax
        jax.devices()
        if device_ids:
            ids = (ctypes.c_int64 * len(device_ids))(*device_ids)
            rc = lib.axon_start_nrt_profile(ids, len(device_ids))
        else:
            rc = lib.axon_start_nrt_profile(None, 0)
        if rc != 0:
            raise RuntimeError(f"axon_start_nrt_profile rc={rc}")
        try:
            yield
        finally:
            n = lib.axon_stop_nrt_profile(str(output_dir).encode())
            if n <= 0:
                print(f"ntff capture wrote {n} files to {output_dir}")

    mod = types.ModuleType("antenv.axon_hooks")
    mod.get_axon_ntff_profile_hook = lambda: _hook
    mod.set_axon_ntff_profile_hook = lambda h: None
    sys.modules["antenv.axon_hooks"] = mod


def _build():
    nc = bacc.Bacc()
    # All inputs pre-arranged: partition dim first, contiguous per partition.
    xH = nc.dram_tensor("xH", [128, B * NCH, KT16, TC], BF16, kind="ExternalInput")
    wqkH = nc.dram_tensor("wqkH", [128, 4, KT16, 128], BF16, kind="ExternalInput")
    wvH = nc.dram_tensor("wvH", [128, KT16, 2 * D], BF16, kind="ExternalInput")
    wpH = nc.dram_tensor("wpH", [128, 2, C], BF16, kind="ExternalInput")
    trid = nc.dram_tensor("trid", [128, 128], BF16, kind="ExternalInput")
    outd = nc.dram_tensor("out", [B * (L // 128), 128, C], BF16, kind="ExternalOutput")

    with tile.TileContext(nc) as tc:
        with tc.tile_pool(name="consts", bufs=1) as cp, \
             tc.tile_pool(name="big", bufs=1) as bp, \
             tc.tile_pool(name="xp", bufs=3) as xp, \
             tc.tile_pool(name="ptp", bufs=8) as ptp, \
             tc.tile_pool(name="smp", bufs=3) as smp, \
             tc.tile_pool(name="dnp", bufs=2) as dnp, \
             tc.tile_pool(name="osp", bufs=3) as osp, \
             tc.tile_pool(name="ps", bufs=3, space="PSUM") as ps, \
             tc.tile_pool(name="pyp", bufs=2, space="PSUM") as pyp, \
             tc.tile_pool(name="prp", bufs=1, space="PSUM") as prp, \
             tc.tile_pool(name="pop", bufs=2, space="PSUM") as pop:

            # PE warm-up on memset data: ramps the clock gate while the first
            # x chunk and weights stream in.
            warm = cp.tile([128, 256], BF16)
            nc.vector.memset(warm, 0.0)
            ones128 = cp.tile([128, 128], BF16)
            nc.vector.memset(ones128, 1.0)
            pw = ps.tile([128, 256], F32, tag="mm")
            for i in range(NWARM):
                nc.tensor.matmul(pw, warm[:, :128], warm,
                                 start=(i == 0), stop=(i == NWARM - 1))

            # Weights ride the sync hwdge queue: m=0 q/k, v, m=1 ahead of
            # chunk 0's x; the rest are emitted behind it (inside the loop).
            wqk = cp.tile([128, 4, KT16, 128], BF16)
            wv = cp.tile([128, KT16, 2 * D], BF16)
            wp = cp.tile([128, 2, C], BF16)
            tri = cp.tile([128, 128], BF16)
            xc0 = xp.tile([128, KT16, TC], BF16, tag="xc", name="xc0")
            nc.sync.dma_start(out=wqk[:, 0], in_=wqkH[:, 0])
            nc.sync.dma_start(out=xc0[:, 0:4], in_=xH[:, 0, 0:4])
            nc.sync.dma_start(out=xc0[:, 4:8], in_=xH[:, 0, 4:8])
            nc.sync.dma_start(out=xc0[:, 8:12], in_=xH[:, 0, 8:12])
            nc.sync.dma_start(out=xc0[:, 12:16], in_=xH[:, 0, 12:16])
            nc.sync.dma_start(out=wqk[:, 1], in_=wqkH[:, 1])
            nc.sync.dma_start(out=wqk[:, 2], in_=wqkH[:, 2])

            # Cross-batch carry of finished-but-unemitted proj pairs.
            pending_c = []
            otile = [None]
            evict_flip = [0]

            def proj_pair(bb, yTref, tt, nch, pool, alt=False):
                po = pool.tile([128, 512], F32, tag="po" if pool is pop else "mm")
                for hi in range(HPC):
                    nc.tensor.matmul(po, yTref[:, hi, tt * 128:(tt + 1) * 128],
                                     wp[:, hi, nch * 512:(nch + 1) * 512],
                                     start=(hi == 0), stop=(hi == HPC - 1))
                if nch == 0:
                    otile[0] = osp.tile([128, C], BF16, tag="ot", name="ot")
                oslice = otile[0][:, nch * 512:(nch + 1) * 512]
                if alt and evict_flip[0] % 2 == 0:
                    nc.scalar.copy(out=oslice, in_=po)
                else:
                    nc.vector.tensor_copy(oslice, po)
                evict_flip[0] += 1
                if nch == 3:
                    q = nc.gpsimd if (tt % 2 == 0) else nc.sync
                    q.dma_start(out=outd[bb * (L // 128) + tt], in_=otile[0])

            for b in range(B):
                # ---- Phase A: q/k/v projections for this batch ----
                QT = bp.tile([128, HPC, L], BF16, tag="QT")   # [d, hi, tok]
                KTt = bp.tile([128, HPC, L], BF16, tag="KT")
                V = bp.tile([128, L // 128, 2 * D], BF16, tag="V")  # [tok, tt, hi*D]
                for ch in range(NCH):
                    if b == 0 and ch == 0:
                        # chunk 0 was DMA'd before the loop; late weights
                        # ride the sync queue behind it
                        xc = xc0
                        nc.sync.dma_start(out=wqk[:, 3], in_=wqkH[:, 3])
                        nc.sync.dma_start(out=wv, in_=wvH[:, :, :])
                        nc.sync.dma_start(out=tri, in_=trid[:, :])
                        nc.sync.dma_start(out=wp, in_=wpH[:, :, :])
                    else:
                        xc = xp.tile([128, KT16, TC], BF16, tag="xc", name="xc")
                        nc.sync.dma_start(out=xc[:, :KT16 // 2],
                                          in_=xH[:, b * NCH + ch, :KT16 // 2])
                        nc.sync.dma_start(out=xc[:, KT16 // 2:],
                                          in_=xH[:, b * NCH + ch, KT16 // 2:])
                    for m in range(4):
                        pq = ps.tile([128, TC], F32, tag="mm")
                        for k in range(KT16):
                            nc.tensor.matmul(pq, wqk[:, m, k], xc[:, k],
                                             start=(k == 0), stop=(k == KT16 - 1))
                        dst = QT if m < 2 else KTt
                        dsl = dst[:, m % 2, ch * TC:(ch + 1) * TC]
                        if m % 2 == 0:
                            nc.scalar.copy(out=dsl, in_=pq)
                        else:
                            nc.vector.tensor_copy(dsl, pq)
                        if pending_c:
                            proj_pair(*pending_c.pop(0), pop, alt=True)
                    for tt in range(TC // 128):
                        # pyp's banks are idle during phase A; using them for
                        # pv gives phase A 4 rotating banks
                        pv = pyp.tile([128, 2 * D], F32, tag="acc", name="pv")
                        for k in range(KT16):
                            nc.tensor.matmul(pv, xc[:, k, tt * 128:(tt + 1) * 128],
                                             wv[:, k], start=(k == 0), stop=(k == KT16 - 1))
                        nc.vector.tensor_copy(V[:, ch * (TC // 128) + tt], pv)
                        if pending_c:
                            proj_pair(*pending_c.pop(0), pop, alt=True)

                # ---- Phase B: causal attention per head ----
                yT = bp.tile([128, HPC, L], BF16, tag="yT")   # [d, hi, tok]
                # Two heads interleaved so the PE always has an independent
                # chain while ACT computes exp for the other head. S^T matmuls
                # run 2 items ahead. Phase C (proj) pairs are injected as PE
                # filler one q-group after their yT inputs were normalized.
                items = [(hi, g, kt)
                         for g in range(4)
                         for hi in range(HPC)
                         for kt in range(4 * (g + 1))]

                def s_matmul(hi, g, kt):
                    off = max(0, 128 * (kt - 4 * g))
                    pss = ps.tile([128, 512], F32, tag="mm")
                    nc.tensor.matmul(pss[:, off:], KTt[:, hi, kt * 128:(kt + 1) * 128],
                                     QT[:, hi, g * 512 + off:(g + 1) * 512],
                                     start=True, stop=True)
                    return pss

                pss_q = [s_matmul(*items[0]), s_matmul(*items[1]),
                         s_matmul(*items[2])]
                psy = {}
                psr = {}
                den = {}
                done_heads = {}
                for i, (hi, g, kt) in enumerate(items):
                    nkt = 4 * (g + 1)
                    off = max(0, 128 * (kt - 4 * g))
                    diag = kt >= 4 * g
                    if kt == 0:
                        psy[hi] = pyp.tile([128, 512], F32, tag="acc", name=f"psy{hi}")
                        psr[hi] = prp.tile([128, 512], F32, tag="rs", name=f"psr{hi}")
                    pss = pss_q.pop(0)
                    if kt == 0:
                        # exp writes straight into the den accumulator; psy
                        # reads the same buffer (saves a DVE copy)
                        den[hi] = dnp.tile([128, 512], BF16, tag=f"den{hi}",
                                           name=f"den{hi}")
                        ptile = den[hi]
                    else:
                        ptile = ptp.tile([128, 512], BF16)
                    nc.scalar.activation(ptile[:, off:], pss[:, off:], AF.Exp, scale=SCALE)
                    if i + 3 < len(items):
                        pss_q.append(s_matmul(*items[i + 3]))
                    if diag and g > 0 and kt == 4 * g:
                        # denominator of the whole non-diag prefix in one matmul
                        nc.tensor.matmul(psr[hi], ones128, den[hi],
                                         start=True, stop=False, skip_group_check=True)
                    if diag:
                        # triangular block of the diagonal k-tile only
                        nc.vector.tensor_mul(ptile[:, off:off + 128],
                                             ptile[:, off:off + 128], tri)
                    nc.tensor.matmul(psy[hi][:, off:], V[:, kt, hi * D:(hi + 1) * D],
                                     ptile[:, off:],
                                     start=(kt == 0), stop=(kt == nkt - 1),
                                     skip_group_check=True)
                    if diag:
                        nc.tensor.matmul(psr[hi][:, off:], ones128, ptile[:, off:],
                                         start=(g == 0 and kt == 0),
                                         stop=(kt == nkt - 1),
                                         skip_group_check=True)
                    elif kt > 0:
                        # running sum of non-diag P tiles off the PE (DVE
                        # bf16 SBUF adds run in 2x mode, ~330ns)
                        nc.vector.tensor_add(den[hi], den[hi], ptile)
                    if kt == nkt - 1:
                        rb = smp.tile([128, 512], F32, tag="rb")
                        nc.vector.reciprocal_approx_fast(out=rb, in_=psr[hi])
                        nc.vector.tensor_mul(yT[:, hi, g * 512:(g + 1) * 512],
                                             psy[hi], rb)
                        done_heads[g] = done_heads.get(g, 0) + 1
                        if done_heads[g] == HPC:
                            pending_c.extend((b, yT, tt, nch)
                                             for tt in range(4 * g, 4 * g + 4)
                                             for nch in range(4))
                    elif pending_c:
                        proj_pair(*pending_c.pop(0), pop, alt=True)
                if b == B - 1:
                    # final drain: alternate the po ring with the (now idle)
                    # pss ring so evictions double-buffer deeper
                    for j, pc in enumerate(pending_c):
                        proj_pair(*pc, pop if j % 2 == 0 else ps, alt=True)
                    pending_c = []
    nc.compile()
    return nc


_cached_nc = None


def kernel(x, w_attn, w_proj):
    global _cached_nc, LAST_RESULT
    if os.environ.get("BASS_TRACE"):
        _install_ntff_shim()
    if _cached_nc is None:
        _cached_nc = _build()
    nc = _cached_nc

    x = np.asarray(x, dtype=np.float32)
    w_attn = np.asarray(w_attn, dtype=np.float32)
    w_proj = np.asarray(w_proj, dtype=np.float32)

    bf16 = ml_dtypes.bfloat16
    # [p, b, ch, k, tc] layout: contiguous 16KB per partition per chunk
    xHa = np.ascontiguousarray(
        x.reshape(B, NCH, TC, KT16, 128).transpose(4, 0, 1, 3, 2)
    ).astype(bf16).reshape(128, B * NCH, KT16, TC)
    kk = np.arange(128)[:, None]
    qq = np.arange(128)[None, :]
    tri = (kk <= qq).astype(bf16)

    in_maps = []
    for c in range(NCORES):
        h0 = HPC * c
        wq = w_attn[h0 * D:(h0 + HPC) * D]
        wk = w_attn[C + h0 * D: C + (h0 + HPC) * D]
        wvv = w_attn[2 * C + h0 * D: 2 * C + (h0 + HPC) * D]
        wqkT = np.concatenate([wq, wk], axis=0).T          # [C, 4D]
        wqkHa = np.ascontiguousarray(
            wqkT.reshape(KT16, 128, 4, 128).transpose(1, 2, 0, 3)
        ).astype(bf16)                                     # [p, m, k, j]
        wvHa = np.ascontiguousarray(
            wvv.T.reshape(KT16, 128, 2 * D).transpose(1, 0, 2)
        ).astype(bf16)                                     # [p, k, vcol]
        wpT = w_proj[:, h0 * D:(h0 + HPC) * D].T           # [2D, C]
        wpHa = np.ascontiguousarray(
            wpT.reshape(2, 128, C).transpose(1, 0, 2)
        ).astype(bf16)                                     # [p, hi, col]
        in_maps.append({
            "xH": xHa,
            "wqkH": wqkHa,
            "wvH": wvHa,
            "wpH": wpHa,
            "trid": tri,
        })

    res = run_bass_kernel_spmd(nc, in_maps, core_ids=list(range(NCORES)))
    LAST_RESULT = res
    acc = res.results[0]["out"].astype(np.float32)
    for i in range(1, NCORES):
        acc += res.results[i]["out"].astype(np.float32)
    return acc.reshape(B, L, C)
